# revision 4
# baseline (speedup 1.0000x reference)
"""ChannelAttention kernel for Trainium2 (8 NeuronCores, batch-parallel).

Reference computation per batch element b (C=64, N=H*W=65536):
    X1 = x[b] viewed [C, N]          (proj_query)
    X2 = x[b] viewed [N, C]          (proj_key -- a reshape, NOT a transpose)
    S  = X1 @ X2                     [C, C]
    P  = softmax(S, axis=-1)
    out[b] = (P @ X1) + X1  =  (P + I) @ X1

Sharding: data-parallel over batch. B=16 -> 2 batches per core on 8 cores.

Per-core dataflow (per batch):
  - x[b] resident in SBUF as 16 column-strips [128, 2048] f32: partition c
    holds X1[c, colhalf0-window], partition 64+c holds X1[c, colhalf1-window].
  - mm1 lhsT tiles: PE-transpose of strip slices [128,128] -> X1^T tiles for
    two n-windows at once (cols 0:64 = window u, cols 64:128 = window u+256).
  - mm1 rhs tiles: X2 contraction-major tiles streamed from HBM with a
    strided access pattern ([128, 32, 64] chunks, 1MB contiguous reads).
  - S accumulates over 512 matmuls in one PSUM tile [64, 64].
  - softmax: DVE row-max (negated) -> ACT exp with fused row-sum ->
    DVE reciprocal -> fused (E * 1/sum) + I.
  - (P+I)^T via PE transpose; replicated to partitions 64:128 via tiny
    SBUF->SBUF DMA so both column-halves of mm2 have aligned operands.
  - mm2: 128 matmuls [64p x 64] @ [64p x 512] -> PSUM -> copy (DVE/ACT
    alternating) into [64, 2048] staging -> 512KB stores to HBM.
"""

import numpy as np

_CACHE = {}

B_FULL = 16
C = 64
N = 65536          # H*W = 256*256
NB = 2             # batches per core
NCORES = 8
NWIN = 256         # 128-col windows per column-half (32768 / 128)
NSTRIP = 16        # strips per batch; strip = [128, 2048]
STRIPW = 2048
VCH = 32           # X2 tiles per V chunk (1 MB per chunk)
NCHUNK = 8         # V chunk pairs per batch (8 * 32 = 256 windows)


def _build():
    import concourse.bacc as bacc
    import concourse.mybir as mybir
    import concourse.tile as tile
    from concourse.masks import make_identity

    f32 = mybir.dt.float32
    Alu = mybir.AluOpType
    Act = mybir.ActivationFunctionType

    nc = bacc.Bacc("TRN2", debug=False)
    xb = nc.dram_tensor("xb", [NB, C * N], f32, kind="ExternalInput").ap()
    ob = nc.dram_tensor("ob", [NB, C * N], f32, kind="ExternalOutput").ap()

    with tile.TileContext(nc) as tc:
        with (
            tc.tile_pool(name="consts", bufs=1) as consts,
            tc.tile_pool(name="H", bufs=NSTRIP) as hpool,
            tc.tile_pool(name="V", bufs=4) as vpool,
            tc.tile_pool(name="TOs", bufs=4) as topool,
            tc.tile_pool(name="stage", bufs=3) as stpool,
            tc.tile_pool(name="soft", bufs=2) as softpool,
            tc.tile_pool(name="psT", bufs=2, space="PSUM") as psT,
            tc.tile_pool(name="psS", bufs=2, space="PSUM") as psS,
            tc.tile_pool(name="psO", bufs=3, space="PSUM") as psO,
            tc.tile_pool(name="psP", bufs=1, space="PSUM") as psP,
        ):
            ident = consts.tile([128, 128], f32)
            make_identity(nc, ident[:])

            for b in range(NB):
                x1 = xb[b].rearrange("(c n) -> c n", c=C)      # [64, 65536]
                o1 = ob[b].rearrange("(c n) -> c n", c=C)

                # ---- load phase: interleave H strips and V chunks ----
                strips = []
                vtiles = []
                for j in range(NCHUNK):
                    for k in (2 * j, 2 * j + 1):
                        st = hpool.tile([128, STRIPW], f32, tag="H")
                        nc.sync.dma_start(
                            st[0:64, :], x1[:, k * STRIPW:(k + 1) * STRIPW]
                        )
                        nc.sync.dma_start(
                            st[64:128, :],
                            x1[:, 32768 + k * STRIPW: 32768 + (k + 1) * STRIPW],
                        )
                        strips.append(st)
                    # V chunk pair: tiles t in [32j, 32j+32) and [256+32j, ...)
                    vts = []
                    for half in range(2):
                        t0 = 256 * half + VCH * j
                        vt = vpool.tile([128, VCH, C], f32, tag="V")
                        src = xb[b][t0 * 8192:(t0 + VCH) * 8192].rearrange(
                            "(t p c) -> p t c", p=128, c=C
                        )
                        nc.sync.dma_start(vt[:], src)
                        vts.append(vt)
                    vtiles.append(vts)

                # ---- mm1: S = X1 @ X2, accumulated over 512 tiles ----
                s_ps = psS.tile([64, 64], f32, tag="S")
                for j in range(NCHUNK):
                    va, vb = vtiles[j]
                    for tl in range(VCH):
                        u = VCH * j + tl             # window index in [0, 256)
                        st = strips[u // 16]         # strip k = u*128//2048
                        ti = st[:, (u % 16) * 128:(u % 16) * 128 + 128]
                        to_ps = psT.tile([128, 128], f32, tag="TO")
                        nc.tensor.transpose(to_ps[:], ti, ident[:])
                        to_sb = topool.tile([128, 128], f32, tag="TOs")
                        if u % 2 == 0:
                            nc.scalar.copy(to_sb[:], to_ps[:])
                        else:
                            nc.vector.tensor_copy(to_sb[:], to_ps[:])
                        nc.tensor.matmul(
                            s_ps[:], to_sb[:, 0:64], va[:, tl, :],
                            start=(u == 0), stop=False,
                        )
                        nc.tensor.matmul(
                            s_ps[:], to_sb[:, 64:128], vb[:, tl, :],
                            start=False, stop=(u == NWIN - 1),
                        )

                # ---- softmax + (P + I), transposed ----
                nmx = softpool.tile([64, 1], f32, tag="nmx")
                nc.vector.tensor_reduce(
                    nmx[:], s_ps[:], axis=mybir.AxisListType.X, op=Alu.max,
                    negate=True,
                )
                esum = softpool.tile([64, 1], f32, tag="esum")
                e_sb = softpool.tile([64, 64], f32, tag="E")
                nc.scalar.activation(
                    e_sb[:], s_ps[:], Act.Exp, bias=nmx[:, 0:1], scale=1.0,
                    accum_out=esum[:],
                )
                rcp = softpool.tile([64, 1], f32, tag="rcp")
                nc.vector.reciprocal(rcp[:], esum[:])
                pi_sb = softpool.tile([64, 64], f32, tag="PI")
                # PI = (E * 1/sum) + I
                nc.vector.scalar_tensor_tensor(
                    pi_sb[:], e_sb[:], rcp[:, 0:1], ident[0:64, 0:64],
                    Alu.mult, Alu.add,
                )
                pit_ps = psP.tile([64, 64], f32, tag="PIT")
                nc.tensor.transpose(pit_ps[:], pi_sb[:], ident[0:64, 0:64])
                pit = softpool.tile([128, 64], f32, tag="PITb")
                nc.vector.tensor_copy(pit[0:64, :], pit_ps[:])
                nc.sync.dma_start(pit[64:128, :], pit[0:64, :])

                # ---- mm2: out = (P+I) @ X1, 128 windows of 512 cols ----
                for half in range(2):
                    lhs = pit[64 * half:64 * half + 64, :]
                    for wg in range(16):          # groups of 4 windows
                        stg = stpool.tile([64, 4, 512], f32, tag="stage")
                        for wi in range(4):
                            w = 4 * wg + wi
                            st = strips[w // 4]
                            rhs = st[64 * half:64 * half + 64,
                                     (w % 4) * 512:(w % 4) * 512 + 512]
                            o_ps = psO.tile([64, 512], f32, tag="O")
                            nc.tensor.matmul(
                                o_ps[:], lhs, rhs, start=True, stop=True
                            )
                            if w % 2 == 0:
                                nc.vector.tensor_copy(stg[:, wi, :], o_ps[:])
                            else:
                                nc.scalar.copy(stg[:, wi, :], o_ps[:])
                        off = 32768 * half + wg * STRIPW
                        nc.scalar.dma_start(
                            o1[:, off:off + STRIPW],
                            stg[:].rearrange("p a b -> p (a b)"),
                        )

    nc.compile()
    return nc


def kernel(x: np.ndarray) -> np.ndarray:
    from concourse.bass_utils import run_bass_kernel_spmd

    if "nc" not in _CACHE:
        _CACHE["nc"] = _build()
    nc = _CACHE["nc"]

    x = np.ascontiguousarray(x, dtype=np.float32)
    B, Cc, H, W = x.shape
    xflat = x.reshape(B, Cc * H * W)
    in_maps = [
        {"xb": xflat[NB * i:NB * (i + 1)]} for i in range(NCORES)
    ]
    res = run_bass_kernel_spmd(nc, in_maps, core_ids=list(range(NCORES)))
    out = np.empty_like(xflat)
    for i in range(NCORES):
        out[NB * i:NB * (i + 1)] = res.results[i]["ob"]
    return out.reshape(B, Cc, H, W)
